# revision 2
# baseline (speedup 1.0000x reference)
"""MoE top-k routing kernel for Trainium2 (nn_MixedOp: top-2 of 8 Dense(1024->1024)+relu, summed).

Strategy:
  - Host: top-k selection over the 8 logits (tiny), slice the k selected expert
    weights/biases, cast x and W to bf16, transpose x so the contraction dim (D)
    is the SBUF partition dim.
  - Device: data-parallel shard of the 8192-token batch across 8 NeuronCores
    (1024 tokens/core), no collectives. Each core computes
        outT[:, t] = sum_e relu(W_e^T @ xT[:, t] + b_e)
    with PE matmuls (bf16 in, fp32 PSUM accumulate), relu+bias fused on the
    scalar engine, expert-sum on the vector engine.
  - Host: transpose per-core outputs back and concatenate.
"""

import os
import sys
from contextlib import ExitStack

if "/opt/trn_rl_repo" not in sys.path:
    sys.path.insert(0, "/opt/trn_rl_repo")

import numpy as np
import ml_dtypes

import concourse.tile as tile
import concourse.bacc as bacc
import concourse.mybir as mybir
from concourse.bass_utils import run_bass_kernel_spmd

NCORES = 8
B = 8192
D = 1024
TPC = B // NCORES      # tokens per core
P = 128                # SBUF partitions
NT = 512               # matmul moving free-dim tile (one fp32 PSUM bank)
DK = D // P            # contraction tiles (8)
EM = D // P            # output-dim tiles (8)
TN = TPC // NT         # token tiles per core (2)

# internal compute dtype: "bf16" | "f32r" | "f32" (f32 native = 4x slower PE)
_DTYPE = os.environ.get("MOE_DTYPE", "bf16")

_nc_cache = {}


def _mdt(dtype: str):
    return {
        "bf16": mybir.dt.bfloat16,
        "f32r": mybir.dt.float32,  # stored f32; bitcast to float32r at matmul
        "f32": mybir.dt.float32,
    }[dtype]


def _npdt(dtype: str):
    return ml_dtypes.bfloat16 if dtype == "bf16" else np.float32


def _build(k: int, dtype: str):
    mdt = _mdt(dtype)
    nc = bacc.Bacc("TRN2", debug=False, target_bir_lowering=False, num_devices=NCORES)
    xT_ap = nc.dram_tensor("xT", [D, TPC], mdt, kind="ExternalInput").ap()
    w_ap = nc.dram_tensor("w", [k, D, D], mdt, kind="ExternalInput").ap()
    bT_ap = nc.dram_tensor("bT", [P, k * EM], mybir.dt.float32, kind="ExternalInput").ap()
    outT_ap = nc.dram_tensor("outT", [D, TPC], mybir.dt.float32, kind="ExternalOutput").ap()

    f32 = mybir.dt.float32

    def mm_ap(t):
        # matmul operand view (float32r bitcast for the fast-fp32 PE mode)
        return t.bitcast(mybir.dt.float32r) if dtype == "f32r" else t

    with tile.TileContext(nc) as tc:
        with ExitStack() as ctx:
            xpool = ctx.enter_context(tc.tile_pool(name="x", bufs=1))
            wpool = ctx.enter_context(tc.tile_pool(name="w", bufs=1))
            bpool = ctx.enter_context(tc.tile_pool(name="b", bufs=1))
            pspool = ctx.enter_context(tc.tile_pool(name="ps", bufs=8, space="PSUM"))
            rpool = ctx.enter_context(tc.tile_pool(name="r", bufs=6))
            opool = ctx.enter_context(tc.tile_pool(name="o", bufs=4))

            bias = bpool.tile([P, k * EM], f32, tag="bias")
            nc.sync.dma_start(out=bias[:], in_=bT_ap[:])

            # resident activations: all of xT for this core (16KB/partition bf16)
            xs = []
            for dk in range(DK):
                t = xpool.tile([P, TPC], mdt, tag=f"x{dk}")
                nc.sync.dma_start(out=t[:], in_=xT_ap[dk * P:(dk + 1) * P, :])
                xs.append(t)

            # resident weights: k experts x DK strips of [128, D]
            ws = {}
            for e in range(k):
                for dk in range(DK):
                    t = wpool.tile([P, D], mdt, tag=f"w{e}_{dk}")
                    nc.sync.dma_start(out=t[:], in_=w_ap[e, dk * P:(dk + 1) * P, :])
                    ws[e, dk] = t

            for em in range(EM):
                rts = []  # [k][TN] relu tiles
                for e in range(k):
                    ps = [
                        pspool.tile([P, NT], f32, name=f"ps_{em}_{e}_{tn}", tag="ps")
                        for tn in range(TN)
                    ]
                    for dk in range(DK):
                        lhsT = ws[e, dk][:, em * P:(em + 1) * P]
                        for tn in range(TN):
                            nc.tensor.matmul(
                                ps[tn][:],
                                mm_ap(lhsT),
                                mm_ap(xs[dk][:, tn * NT:(tn + 1) * NT]),
                                start=(dk == 0),
                                stop=(dk == DK - 1),
                            )
                    rr = []
                    for tn in range(TN):
                        r = rpool.tile([P, NT], f32)
                        nc.scalar.activation(
                            r[:], ps[tn][:], mybir.ActivationFunctionType.Relu,
                            bias=bias[:, e * EM + em: e * EM + em + 1],
                        )
                        rr.append(r)
                    rts.append(rr)
                for tn in range(TN):
                    dst = outT_ap[em * P:(em + 1) * P, tn * NT:(tn + 1) * NT]
                    if k == 1:
                        nc.sync.dma_start(out=dst, in_=rts[0][tn][:])
                    else:
                        o = opool.tile([P, NT], f32)
                        nc.vector.tensor_add(o[:], rts[0][tn][:], rts[1][tn][:])
                        for e in range(2, k):
                            nc.vector.tensor_add(o[:], o[:], rts[e][tn][:])
                        nc.sync.dma_start(out=dst, in_=o[:])

    nc.compile()
    return nc


def _get_nc(k: int, dtype: str):
    key = (k, dtype)
    if key not in _nc_cache:
        _nc_cache[key] = _build(k, dtype)
    return _nc_cache[key]


def _prep_in_maps(x, logits, Ws, bs, k, dtype):
    x = np.asarray(x, dtype=np.float32)
    logits = np.asarray(logits, dtype=np.float32)
    Ws = np.asarray(Ws, dtype=np.float32)
    bs = np.asarray(bs, dtype=np.float32)

    # top-k by logits, descending, ties -> lower index (matches jax.lax.top_k)
    ids = np.argsort(-logits, kind="stable")[:k]

    npdt = _npdt(dtype)
    Wd = np.ascontiguousarray(Ws[ids].astype(npdt))              # [k, D, D]
    bT = np.ascontiguousarray(
        bs[ids].reshape(k, EM, P).transpose(2, 0, 1).reshape(P, k * EM)
    ).astype(np.float32)                                         # [P, k*EM]
    xT = x.astype(npdt).T                                        # [D, B] view

    in_maps = []
    for c in range(NCORES):
        in_maps.append({
            "xT": np.ascontiguousarray(xT[:, c * TPC:(c + 1) * TPC]),
            "w": Wd,
            "bT": bT,
        })
    return in_maps


def _gather(results):
    out = np.empty((B, D), dtype=np.float32)
    for c in range(NCORES):
        out[c * TPC:(c + 1) * TPC, :] = results[c]["outT"].T
    return out


def kernel(x, logits, Ws, bs, num_on_samples):
    k = int(num_on_samples)
    in_maps = _prep_in_maps(x, logits, Ws, bs, k, _DTYPE)
    nc = _get_nc(k, _DTYPE)
    res = run_bass_kernel_spmd(nc, in_maps, list(range(NCORES)))
    return _gather(res.results)


def run_traced(x, logits, Ws, bs, num_on_samples, dtype=None, **spmd_kwargs):
    """Dev helper: same as kernel() but returns (output, BassKernelResults)."""
    k = int(num_on_samples)
    dtype = dtype or _DTYPE
    in_maps = _prep_in_maps(x, logits, Ws, bs, k, dtype)
    nc = _get_nc(k, dtype)
    res = run_bass_kernel_spmd(nc, in_maps, list(range(NCORES)), **spmd_kwargs)
    return _gather(res.results), res
